# revision 30
# baseline (speedup 1.0000x reference)
"""Trainium2 Bass kernel for a dense transformer block (B=4, T=2048, C=1024, H=16).

Sharding: data-parallel over tokens. Core i owns batch b=i//2, token-half i%2
(1024 tokens). Each core redundantly computes LN1/K/V for its batch's full 2048
tokens so there are no collectives at all.

v2: fp8 attention path + bf16 MLP.
  - QKV / proj use fp8e4 DoubleRow matmuls (contract 256/instr): ~1.7x fewer
    PE cycles on those GEMMs. MLP stays bf16 (fp8 there costs ~2.5e-2 rel err,
    over the 2e-2 budget; attention-side fp8 measures ~2.7e-3 total).
  - scores contract fp8 k/q (same PE speed as bf16, halves SBUF for k/q).
  - AV contracts keys 2048-deep: fp8 DoubleRow over key-tile pairs; exp is
    emitted straight to fp8 with a 2^-3 shift (softmax-invariant) so et stays
    far from the fp8e4 +-240 range.
  - softmax denominator rides the AV matmul as a 65th all-ones column of V;
    1/den via reciprocal_approx_fast (5x faster than DVE reciprocal), then
    gpsimd partition-broadcast.
  - V stays SBUF-resident in AV layout (fp8 = 16.6KB/partition); no DRAM
    staging roundtrip.
  - schedule: QKV phase -> attention(qc0) with V/Q tail matmuls interleaved
    into exp gaps -> attention(qc1) with MLP1(qc0) interleaved -> MLP tail.
    PE work is fed from a background-task deque popped between score groups,
    so the PE stays busy while ACT (the attention bottleneck) runs exp.
PSUM: tags d2 [128,2,512] bufs=2 (scores double-buffer / stats / MLP acc),
a1 [128,512] bufs=2 (AV accum / MLP acc), m1 [128,512] bufs=2 (bg-task acc).
LN scale (g) is folded into weight rows and LN shift (beta) into biases on the
host, so on-chip LN is a pure normalize.
"""

import sys

if "/opt/trn_rl_repo" not in sys.path:
    sys.path.insert(0, "/opt/trn_rl_repo")

from collections import deque

import numpy as np
import ml_dtypes

B, T, C, H, HD = 4, 2048, 1024, 16, 64
FF = 4 * C
TO = T // 2          # tokens owned per core
NCC = C // 128       # 8
NP = 4               # DoubleRow pairs over C
NFC = FF // 128      # 32
EPS = 1e-5
SCALE = C ** -0.5    # 1/32
EBIAS = float(-3.0 * np.log(2.0))   # exp shift: et = exp(s/32)/8 (softmax-inv)
BF16 = ml_dtypes.bfloat16
FP8 = ml_dtypes.float8_e4m3

_BUILT = None


def _emit(nc, tc, aps, has_bias, has_bv):
    from concourse import mybir
    from concourse.bass import ts
    F32 = mybir.dt.float32
    BF = mybir.dt.bfloat16
    F8 = mybir.dt.float8e4
    AF = mybir.ActivationFunctionType
    ADD = mybir.AluOpType.add

    DR = mybir.MatmulPerfMode.DoubleRow
    from contextlib import ExitStack

    xT, wq, wk, wv, wproj, w1, w2, x2d, outT = (
        aps["xT"], aps["wq"], aps["wk"], aps["wv"], aps["wproj"], aps["w1"],
        aps["w2"], aps["x2d"], aps["outT"])

    ctx = ExitStack()
    with ctx:
        const = ctx.enter_context(tc.tile_pool(name="const", bufs=1))
        wts = ctx.enter_context(tc.tile_pool(name="wts", bufs=1))
        attn = ctx.enter_context(tc.tile_pool(name="attn", bufs=1))
        misc = ctx.enter_context(tc.tile_pool(name="misc", bufs=2))
        wpool = ctx.enter_context(tc.tile_pool(name="wpool", bufs=8))
        psum = ctx.enter_context(tc.tile_pool(name="psum", bufs=1, space="PSUM"))

        def ps_d2():
            return psum.tile([128, 2, 512], F32, name="ps_d2", tag="d2", bufs=1)

        def ps_q4():
            return psum.tile([128, 4, 512], F32, name="ps_q4", tag="q4", bufs=1)

        def ps_a1():
            return psum.tile([128, 512], F32, name="ps_a1", tag="a1", bufs=2)

        def ps_m1():
            return psum.tile([128, 512], F32, name="ps_m1", tag="m1", bufs=2)

        def _rot_gen():
            # rotation of [128,512] psum accumulator views for dense GEMM phases
            while True:
                t = ps_q4()
                for j in range(4):
                    yield t[:, j, :]
                yield ps_a1()
                yield ps_m1()

        _rot = _rot_gen()

        def ps_rot():
            return next(_rot)

        # ---------------- constants / biases ----------------
        ones_sc = const.tile([128, 128], BF, name="ones_sc")
        nc.vector.memset(ones_sc, 1.0 / C)
        # fp8 ones for the mean matmul over fp8 x tiles (1/C is subnormal in
        # fp8e4; use 1/8 and rescale the mean by 8/C downstream)
        ones_f8 = const.tile([128, 128], F8, name="ones_f8")
        nc.vector.memset(ones_f8, 0.125)
        eb_sb = const.tile([128, 1], F32, name="eb_sb")
        nc.vector.memset(eb_sb, EBIAS)
        eps_sb = const.tile([128, 1], F32, name="eps_sb")
        nc.vector.memset(eps_sb, EPS)
        if has_bias:
            bq_sb = const.tile([128, 8], F32, name="bq_sb")
            bk_sb = const.tile([128, 8], F32, name="bk_sb")
            bp_sb = const.tile([128, 8], F32, name="bp_sb")
            b2_sb = const.tile([128, 8], F32, name="b2_sb")
            b1_sb = const.tile([128, 32], F32, name="b1_sb")
            nc.sync.dma_start(out=bq_sb, in_=aps["bq"])
            nc.sync.dma_start(out=bk_sb, in_=aps["bk"])
            nc.sync.dma_start(out=bp_sb, in_=aps["bp"])
            nc.sync.dma_start(out=b2_sb, in_=aps["b2c"])
            nc.sync.dma_start(out=b1_sb, in_=aps["b1c"])
        else:
            bp_sb = const.tile([128, 8], F32, name="bp_sb")
            nc.vector.memset(bp_sb, 0.0)
            b2_sb = bp_sb
            b1_sb = const.tile([128, 32], F32, name="b1_sb")
            nc.vector.memset(b1_sb, 0.0)
        if has_bv:
            bv_sb = const.tile([1, 1024], F32, name="bv_sb")
            nc.sync.dma_start(out=bv_sb, in_=aps["bvrow"])
            bvb = const.tile([128, 1024], F32, name="bvb")
            nc.gpsimd.partition_broadcast(bvb, bv_sb)

        # ---------------- persistent weights (fp8 DR layout) ----------------
        # wk/wv/wq rotate through 3 buffers; wproj later takes wk's slot
        # (wk is only read in phase B). DMAs are emitted in phase B, after
        # the first x chunk's loads (x gates the pipeline start).
        def wtile():
            return wts.tile([128, NP, 2, 1024], F8, name="w4", tag="w4", bufs=3)

        wk_sb, wv_sb, wq_sb = wtile(), wtile(), wtile()

        # ---------------- persistent attention tensors ----------------
        k_sb = attn.tile([128, 8, T], F8, name="k_sb")
        q_sb = attn.tile([128, 8, TO], F8, name="q_sb")
        # v_aug[key_r, kgroup, pair, head, 65]: key = 256*kgroup + 128*pair + r
        v_aug = attn.tile([128, 8, 2, 16, 65], F8, name="v_aug")
        o_sb = attn.tile([128, 8, 512], F8, name="o_sb")
        z0_sb = attn.tile([128, NFC, 512], BF, name="z_sb", tag="z", bufs=1)
        nc.vector.memset(v_aug[:, :, :, :, 64:65], 1.0)

        # h tiles (LN1 output, fp8) for all 4 token chunks
        hts = []

        def stats_finish(st, tag, bufs=2, mean_scale=1.0):
            """st: psum [:,0]=mean/mean_scale, [:,1]=E[x^2] (rows replicated).
            Returns (mu_sb, s_sb) sbuf f32 tiles."""
            mu_sb = misc.tile([128, 512], F32, name="mu_sb", tag=tag, bufs=bufs)
            nc.scalar.mul(out=mu_sb, in_=st[:, 0, :], mul=mean_scale)
            musq = misc.tile([128, 512], F32, name="musq", tag="stat", bufs=2)
            nc.scalar.activation(out=musq, in_=st[:, 0, :], func=AF.Square,
                                 scale=mean_scale)
            var = misc.tile([128, 512], F32, name="var", tag="stat", bufs=2)
            nc.vector.tensor_sub(out=var, in0=st[:, 1, :], in1=musq)
            sd = misc.tile([128, 512], F32, name="sd", tag="stat", bufs=2)
            nc.scalar.activation(out=sd, in_=var, func=AF.Sqrt, bias=eps_sb)
            s_sb = misc.tile([128, 512], F32, name="s_sb", tag=tag, bufs=bufs)
            nc.vector.reciprocal_approx_fast(out=s_sb, in_=sd)
            return mu_sb, s_sb

        def ln_stats(tcg):
            """Load + cast one 512-token chunk, emit its LN1 stats matmuls.
            x is cast to fp8 (same noise as the later fp8 h cast); the mean
            matmul contracts fp8 with 1/8 ones, rescaled in stats_finish."""
            st = ps_d2()
            xq = []
            for c in range(NCC):
                xs = misc.tile([128, 512], F32, name="xs", tag="xs", bufs=3)
                nc.sync.dma_start(out=xs, in_=xT[c, :, ts(tcg, 512)])
                xqc = misc.tile([128, 512], F8, name="xqc", tag="xk", bufs=16)
                nc.scalar.copy(out=xqc, in_=xs)
                xq.append(xqc)
                xsq = misc.tile([128, 512], BF, name="xsq", tag="bfts", bufs=3)
                nc.vector.tensor_mul(out=xsq, in0=xs, in1=xs)
                nc.tensor.matmul(st[:, 0, :], ones_f8, xqc,
                                 start=(c == 0), stop=(c == NCC - 1),
                                 skip_group_check=True)
                nc.tensor.matmul(st[:, 1, :], ones_sc, xsq,
                                 start=(c == 0), stop=(c == NCC - 1),
                                 skip_group_check=True)
            return st, xq

        def ln_apply(st, xq, h_dst):
            mu_sb, s_sb = stats_finish(st, "mstat", mean_scale=8.0 / C)
            for c in range(NCC):
                d = misc.tile([128, 512], F32, name="d", tag="xs", bufs=3)
                nc.vector.tensor_sub(out=d, in0=xq[c], in1=mu_sb)
                nc.vector.tensor_mul(out=h_dst[:, c, :], in0=d, in1=s_sb)

        def evac(dst, src, bias_col):
            """psum -> sbuf evacuation on ACT (idle during the QKV phase,
            where these run; keeps DVE free for the LN chains)."""
            if bias_col is None:
                nc.scalar.copy(out=dst, in_=src)
            else:
                nc.scalar.activation(out=dst, in_=src, func=AF.Identity,
                                     bias=bias_col)

        # ---------------- DR matmul emitters ----------------
        # acc: psum accumulator source. Phase B uses the full rotation; bg
        # tasks run inside attention and must stay off the scores (d2) and
        # AV (a1) banks, so they pass ps_m1.
        def emit_k_tile(m, tcg, acc=None):
            """K features m*128.. for token chunk tcg -> k_sb[:, m, tcg]."""
            kp = (acc or ps_rot)()
            for p in range(NP):
                nc.tensor.matmul(kp, wk_sb[:, p, :, ts(m, 128)],
                                 hts[tcg][:, 2 * p:2 * p + 2, :],
                                 start=(p == 0), stop=(p == NP - 1),
                                 perf_mode=DR, skip_group_check=True)
            evac(k_sb[:, m, ts(tcg, 512)], kp,
                 bk_sb[:, m:m + 1] if has_bias else None)

        def emit_q_tile(m, tcg, acc=None):
            """Q features m*128.. for own-token chunk tcg (0/1) -> qz0/qz1."""
            qp = (acc or ps_rot)()
            for p in range(NP):
                nc.tensor.matmul(qp, wq_sb[:, p, :, ts(m, 128)],
                                 hts[tcg][:, 2 * p:2 * p + 2, :],
                                 start=(p == 0), stop=(p == NP - 1),
                                 perf_mode=DR, skip_group_check=True)
            evac(q_sb[:, m, ts(tcg, 512)], qp,
                 bq_sb[:, m:m + 1] if has_bias else None)

        def emit_v_tile(tcg, tt, nch, acc=None):
            """V for 128 tokens (key-tile kt=tcg*4+tt), 8 heads (nch half)."""
            vp = (acc or ps_rot)()
            for p in range(NP):
                nc.tensor.matmul(vp, hts[tcg][:, 2 * p:2 * p + 2, ts(tt, 128)],
                                 wv_sb[:, p, :, ts(nch, 512)],
                                 start=(p == 0), stop=(p == NP - 1),
                                 perf_mode=DR, skip_group_check=True)
            kt = tcg * 4 + tt
            gg, ii = kt // 2, kt % 2
            dst = v_aug[:, gg, ii, 8 * nch:8 * nch + 8, 0:64]
            src = vp.rearrange("p (h d) -> p h d", h=8)
            if has_bv:
                bslice = bvb[:, ts(nch, 512)].rearrange("p (h d) -> p h d", h=8)
                nc.vector.tensor_add(out=dst, in0=src, in1=bslice)
            else:
                nc.scalar.copy(out=dst, in_=src)

        # background PE-work queue: popped between attention score groups
        bg = deque()

        def run_bg(n):
            for _ in range(n):
                if not bg:
                    return
                bg.popleft()()

        # ---------------- phase B: LN1 + K + V(half) + Q(qc0 half) ----------
        # Stats of chunk tcg+1 are emitted before K/V of chunk tcg so the
        # stats_finish chain (ACT/DVE, ~4us) hides under K/V matmuls. The
        # first x-chunk DMAs precede the weight DMAs (x gates the pipeline).
        st_xq = [None] * 4
        st_xq[0] = ln_stats(0)
        for p in range(NP):
            nc.sync.dma_start(out=wk_sb[:, p], in_=wk[p])
            nc.sync.dma_start(out=wv_sb[:, p], in_=wv[p])
        for p in range(NP):
            nc.sync.dma_start(out=wq_sb[:, p], in_=wq[p])
        for tcg in range(4):
            if tcg + 1 < 4:
                st_xq[tcg + 1] = ln_stats(tcg + 1)
            h_t = attn.tile([128, 8, 512], F8, name="h_t", tag="h", bufs=4)
            hts.append(h_t)
            ln_apply(*st_xq[tcg], h_t)
            st_xq[tcg] = None
            for m in range(NCC):
                emit_k_tile(m, tcg)
            for tt in range(4):
                emit_v_tile(tcg, tt, 0)
        for tcg in range(4):
            for tt in range(2):
                emit_v_tile(tcg, tt, 1)
        for m in range(4):
            emit_q_tile(m, 0)
            emit_q_tile(m, 1)

        # deferred into attention(qc0) gaps, deadline order: all V(nch=1)
        # leftovers and qz0 chunk m are first read at head 8; qz1 leftovers
        # only in attention(qc1).
        for tcg in range(4):
            for tt in (2, 3):
                bg.append(lambda tcg=tcg, tt=tt: emit_v_tile(tcg, tt, 1, ps_m1))
        for m in range(4, NCC):
            bg.append(lambda m=m: emit_q_tile(m, 0, ps_m1))
        for m in range(4, NCC):
            bg.append(lambda m=m: emit_q_tile(m, 1, ps_m1))

        # ---------------- attention + proj + fused LN2 stats ----------------
        ln2_stats = {}
        h2s = {}

        def attn_head_pair(hp, qc, bg_slots=()):
            """Head pair 2hp/2hp+1. Scores contract K=64 as two row-tiled
            matmuls at partition bases 0/64 that run concurrently in the PE
            (per-subarray row groups) -- no zero-padded q, half the score
            cycles. One exp covers the 4-bank group [A-j0, A-j1, B-j0, B-j1]
            (2048 cols/call); AV picks each head's contiguous bank pair with
            fp8 DoubleRow. bg tasks fill PE slack while ACT runs exp."""
            hA, hB = 2 * hp, 2 * hp + 1
            avpA = ps_a1()
            avpB = ps_a1()

            def emit_scores(g):
                scp = ps_q4()
                for j in range(2):
                    sk = g * 2 + j
                    nc.tensor.matmul(scp[:, j, :], k_sb[0:64, hp, ts(sk, 128)],
                                     q_sb[0:64, hp, ts(qc, 512)],
                                     start=True, stop=True,
                                     tile_position=(0, 0),
                                     skip_group_check=True)
                    nc.tensor.matmul(scp[:, 2 + j, :],
                                     k_sb[64:128, hp, ts(sk, 128)],
                                     q_sb[64:128, hp, ts(qc, 512)],
                                     start=True, stop=True,
                                     tile_position=(64, 0),
                                     skip_group_check=True)
                return scp

            scp = emit_scores(0)
            for g in range(8):
                et = misc.tile([128, 4, 512], F8, name="et", tag="et", bufs=2)
                nc.scalar.activation(out=et, in_=scp, func=AF.Exp,
                                     scale=SCALE, bias=eb_sb)
                if g in bg_slots:
                    run_bg(1)
                nc.tensor.matmul(avpA[0:65, :], v_aug[:, g, :, hA, :],
                                 et[:, 0:2, :], start=(g == 0), stop=(g == 7),
                                 perf_mode=DR, skip_group_check=True)
                nc.tensor.matmul(avpB[0:65, :], v_aug[:, g, :, hB, :],
                                 et[:, 2:4, :], start=(g == 0), stop=(g == 7),
                                 perf_mode=DR, skip_group_check=True)
                if g + 1 < 8:
                    scp = emit_scores(g + 1)
            for h, avp in ((hA, avpA), (hB, avpB)):
                # reciprocal_approx_fast misreads PSUM sources on HW (bitwise
                # seed trick) -- bounce the denominator row through SBUF.
                den = misc.tile([1, 512], F32, name="den", tag="den", bufs=2)
                nc.vector.tensor_copy(out=den, in_=avp[64:65, :])
                r_t = misc.tile([1, 512], F32, name="r_t", tag="r", bufs=2)
                nc.vector.reciprocal_approx_fast(out=r_t, in_=den)
                rb_t = misc.tile([64, 512], F32, name="rb_t", tag="rb", bufs=2)
                nc.gpsimd.partition_broadcast(rb_t, r_t)
                p0 = (h % 2) * 64
                nc.vector.tensor_mul(out=o_sb[p0:p0 + 64, hp, :],
                                     in0=avp[0:64, :], in1=rb_t)

        def emit_proj_tile(m, qc, st2):
            pp = ps_m1()
            for p in range(NP):
                nc.tensor.matmul(pp, wp_sb[:, p, :, ts(m, 128)],
                                 o_sb[:, 2 * p:2 * p + 2, :],
                                 start=(p == 0), stop=(p == NP - 1),
                                 perf_mode=DR, skip_group_check=True)
            xres = misc.tile([128, 512], F32, name="xres", tag="xres", bufs=2)
            nc.sync.dma_start(out=xres, in_=xT[m, :, ts(qc, 512)])
            x2t = misc.tile([128, 512], F32, name="x2t", tag="x2t", bufs=2)
            nc.vector.scalar_tensor_tensor(out=x2t, in0=pp,
                                           scalar=bp_sb[:, m:m + 1],
                                           in1=xres, op0=ADD, op1=ADD)
            nc.sync.dma_start(out=x2d[m, :, ts(qc, 512)], in_=x2t)
            xb2 = misc.tile([128, 512], BF, name="xb2", tag="bfts", bufs=3)
            nc.scalar.copy(out=xb2, in_=x2t)
            xq2 = misc.tile([128, 512], BF, name="xq2", tag="bfts", bufs=3)
            nc.vector.tensor_mul(out=xq2, in0=x2t, in1=x2t)
            nc.tensor.matmul(st2[:, 0, :], ones_sc, xb2,
                             start=(m == 0), stop=(m == NCC - 1),
                             skip_group_check=True)
            nc.tensor.matmul(st2[:, 1, :], ones_sc, xq2,
                             start=(m == 0), stop=(m == NCC - 1),
                             skip_group_check=True)

        def emit_h2_chunk(c, qc, mu2, s2, h2):
            xs2 = misc.tile([128, 512], F32, name="xs2", tag="xs", bufs=3)
            nc.sync.dma_start(out=xs2, in_=x2d[c, :, ts(qc, 512)])
            d2t = misc.tile([128, 512], F32, name="d2t", tag="xs", bufs=3)
            nc.vector.tensor_sub(out=d2t, in0=xs2, in1=mu2)
            nc.vector.tensor_mul(out=h2[:, c, :], in0=d2t, in1=s2)

        def emit_mlp1_tile(m, qc, z_sb):
            """z[:, m, :] = relu(W1 block @ h2) (+bias). w1 is laid out
            [32 m][8 k][128][128] so each tile loads exactly its own slice."""
            acc = ps_m1()
            h2 = h2s[qc]
            for k in range(NCC):
                w1t = wpool.tile([128, 128], BF, name="w1t", tag="w1s", bufs=8)
                nc.sync.dma_start(out=w1t, in_=w1[m, k])
                nc.tensor.matmul(acc, w1t, h2[:, k, :],
                                 start=(k == 0), stop=(k == NCC - 1),
                                 skip_group_check=True)
            nc.scalar.activation(out=z_sb[:, m, :], in_=acc,
                                 func=AF.Relu, bias=b1_sb[:, m:m + 1])

        # --- attention qc0: V/Q leftovers fill exp gaps ---
        # wproj takes over wk's buffer; its DMA overlaps attention(qc0).
        wp_sb = wtile()
        for p in range(NP):
            nc.sync.dma_start(out=wp_sb[:, p], in_=wproj[p])
        for hp in range(H // 2):
            attn_head_pair(hp, 0, bg_slots=(1, 3, 5) if hp < 4 else (2, 5))
        st2 = ps_d2()
        for m in range(NCC):
            emit_proj_tile(m, 0, st2)
        ln2_stats[0] = stats_finish(st2, "mstat")
        h2_0 = attn.tile([128, 8, 512], BF, name="h2_0")
        h2s[0] = h2_0
        mu2, s2 = ln2_stats[0]
        for c in range(NCC):
            emit_h2_chunk(c, 0, mu2, s2, h2_0)

        # --- attention qc1: MLP1(qc0) fills exp gaps ---
        # w1 layout is [mg 8][k 8][128][512]; a tile covers out features
        # mg*512+m4*128. Queue MLP1(qc0) as bg tasks.
        for m in range(NFC):
            bg.append(lambda m=m: emit_mlp1_tile(m, 0, z0_sb))
        for hp in range(H // 2):
            attn_head_pair(hp, 1, bg_slots=(1, 3, 5, 7))
        st2b = ps_d2()
        for m in range(NCC):
            emit_proj_tile(m, 1, st2b)
        ln2_stats[1] = stats_finish(st2b, "mstat")
        h2_1 = attn.tile([128, 8, 512], BF, name="h2_1")
        h2s[1] = h2_1
        mu2b, s2b = ln2_stats[1]
        for c in range(NCC):
            emit_h2_chunk(c, 1, mu2b, s2b, h2_1)

        # ---------------- MLP tail ----------------
        run_bg(len(bg))  # drain any MLP1(qc0) leftovers

        def mlp2_wave(qc, z_sb):
            """out = W2 @ z + b2 + x2 for one 512-token chunk; all 8 output
            feature tiles accumulate at once (8 psum banks), each w2 row tile
            loaded once per wave."""
            t_q = ps_q4()
            accs = [t_q[:, 0, :], t_q[:, 1, :], t_q[:, 2, :], t_q[:, 3, :],
                    ps_a1(), ps_a1(), ps_m1(), ps_m1()]
            for k in range(NFC):
                w2t = wpool.tile([128, 1024], BF, name="w2t", tag="w2s", bufs=4)
                nc.sync.dma_start(out=w2t, in_=w2[k])
                for m in range(8):
                    nc.tensor.matmul(accs[m], w2t[:, ts(m, 128)], z_sb[:, k, :],
                                     start=(k == 0), stop=(k == NFC - 1),
                                     skip_group_check=True)
            for m in range(8):
                xr2 = misc.tile([128, 512], F32, name="xr2", tag="xr2", bufs=2)
                nc.sync.dma_start(out=xr2, in_=x2d[m, :, ts(qc, 512)])
                ot = misc.tile([128, 512], F32, name="ot", tag="ot", bufs=2)
                nc.vector.scalar_tensor_tensor(out=ot, in0=accs[m],
                                               scalar=b2_sb[:, m:m + 1],
                                               in1=xr2, op0=ADD, op1=ADD)
                nc.sync.dma_start(out=outT[m, :, ts(qc, 512)], in_=ot)

        mlp2_wave(0, z0_sb)
        z1_sb = attn.tile([128, NFC, 512], BF, name="z_sb", tag="z", bufs=1)
        for m in range(NFC):
            emit_mlp1_tile(m, 1, z1_sb)
        mlp2_wave(1, z1_sb)


def _build(has_bias, has_bv):
    from concourse import bacc, mybir, tile
    F32 = mybir.dt.float32
    BF = mybir.dt.bfloat16
    F8 = mybir.dt.float8e4

    nc = bacc.Bacc("TRN2", target_bir_lowering=False, debug=False,
                   enable_asserts=False, num_devices=8)
    aps = {}
    aps["xT"] = nc.dram_tensor("xT", [8, 128, T], F32, kind="ExternalInput").ap()
    for n in ("wq", "wk", "wv", "wproj"):
        aps[n] = nc.dram_tensor(n, [NP, 128, 2, 1024], F8,
                                kind="ExternalInput").ap()
    aps["w1"] = nc.dram_tensor("w1", [NFC, 8, 128, 128], BF,
                               kind="ExternalInput").ap()
    aps["w2"] = nc.dram_tensor("w2", [NFC, 128, 1024], BF,
                               kind="ExternalInput").ap()
    if has_bias:
        for n in ("bq", "bk", "bp", "b2c"):
            aps[n] = nc.dram_tensor(n, [128, 8], F32, kind="ExternalInput").ap()
        aps["b1c"] = nc.dram_tensor("b1c", [128, 32], F32,
                                    kind="ExternalInput").ap()
    if has_bv:
        aps["bvrow"] = nc.dram_tensor("bvrow", [1, 1024], F32,
                                      kind="ExternalInput").ap()
    aps["x2d"] = nc.dram_tensor("x2d", [8, 128, TO], F32).ap()
    aps["outT"] = nc.dram_tensor("outT", [8, 128, TO], F32,
                                 kind="ExternalOutput").ap()

    with tile.TileContext(nc) as tcx:
        _emit(nc, tcx, aps, has_bias, has_bv)
    nc.compile()
    return nc


def _prep_inputs(x, Wq, Wk, Wv, Wproj, bproj, W1, b1, W2, b2, g1, be1, g2, be2):
    """Host-side prep: fold LN affine into weights/biases, cast, lay out."""
    x = np.asarray(x, np.float32)
    g1 = np.asarray(g1, np.float32)
    be1 = np.asarray(be1, np.float32)
    g2 = np.asarray(g2, np.float32)
    be2 = np.asarray(be2, np.float32)

    def to2d(w):  # (H, C, hd) -> (C, H*hd)
        return np.asarray(w, np.float32).transpose(1, 0, 2).reshape(C, C)

    wq2, wk2, wv2 = to2d(Wq), to2d(Wk), to2d(Wv)
    Wproj = np.asarray(Wproj, np.float32)
    W1 = np.asarray(W1, np.float32)
    W2 = np.asarray(W2, np.float32)

    wq_e, wk_e, wv_e = g1[:, None] * wq2, g1[:, None] * wk2, g1[:, None] * wv2
    w1_e = g2[:, None] * W1
    bias_q = be1 @ wq2
    bias_k = be1 @ wk2
    bias_v = be1 @ wv2
    bias_1 = np.asarray(b1, np.float32) + be2 @ W1

    def wdr(w):  # (C, N) -> (NP, 128, 2, N) fp8 DoubleRow pair layout
        # contraction index = 256*p + 128*i + r
        return np.ascontiguousarray(
            w.reshape(NP, 2, 128, -1).transpose(0, 2, 1, 3).astype(FP8))

    def bvec(v):  # (N,) -> (128, N//128) partition-major
        return np.ascontiguousarray(np.asarray(v, np.float32).reshape(-1, 128).T)

    shared = {
        "wq": wdr(wq_e), "wk": wdr(wk_e), "wv": wdr(wv_e), "wproj": wdr(Wproj),
        "w1": np.ascontiguousarray(
            w1_e.reshape(NCC, 128, NFC, 128).transpose(2, 0, 1, 3).astype(BF16)),
        "w2": np.ascontiguousarray(W2.reshape(NFC, 128, C).astype(BF16)),
    }
    bias_p = np.asarray(bproj, np.float32)
    bias_2 = np.asarray(b2, np.float32)
    has_bias = bool(np.any(bias_q != 0) or np.any(bias_k != 0)
                    or np.any(bias_p != 0) or np.any(bias_1 != 0)
                    or np.any(bias_2 != 0))
    if has_bias:
        shared.update({
            "bq": bvec(bias_q), "bk": bvec(bias_k), "bp": bvec(bias_p),
            "b2c": bvec(bias_2), "b1c": bvec(bias_1),
        })
    has_bv = bool(np.any(bias_v != 0.0))
    if has_bv:
        shared["bvrow"] = np.ascontiguousarray(bias_v.reshape(1, C))

    in_maps = []
    for core in range(8):
        b, half = core // 2, core % 2
        xt = x[b].T  # (C, T)
        own = xt[:, half * TO:(half + 1) * TO]
        oth = xt[:, (1 - half) * TO:(2 - half) * TO]
        m = dict(shared)
        m["xT"] = np.ascontiguousarray(
            np.concatenate([own, oth], axis=1).reshape(NCC, 128, T))
        in_maps.append(m)
    return in_maps, has_bias, has_bv


def kernel(x, Wq, Wk, Wv, Wproj, bproj, W1, b1, W2, b2, g1, be1, g2, be2):
    global _BUILT
    from concourse.bass_utils import run_bass_kernel_spmd

    in_maps, has_bias, has_bv = _prep_inputs(
        x, Wq, Wk, Wv, Wproj, bproj, W1, b1, W2, b2, g1, be1, g2, be2)
    if _BUILT is None or _BUILT[1] != (has_bias, has_bv):
        _BUILT = (_build(has_bias, has_bv), (has_bias, has_bv))
    nc = _BUILT[0]
    res = run_bass_kernel_spmd(nc, in_maps, core_ids=list(range(8)))
    out = np.empty((B, T, C), np.float32)
    for core in range(8):
        b, half = core // 2, core % 2
        o = res.results[core]["outT"].reshape(C, TO)  # (feature, token)
        out[b, half * TO:(half + 1) * TO, :] = o.T
    return out
